# revision 31
# baseline (speedup 1.0000x reference)
"""Trainium2 Bass kernel for nn_Decomp_Forecast (HiPPO-LegS decomposition forecaster).

Math: the reference runs a 720-step linear scan c_t = c_{t-1} @ A^T + f_t * B
and only uses the final state, so the whole model collapses (exactly, by
associativity) to two chained matmuls around the instance-norm statistics:

    G[t]   = B^T (A^T)^(T-1-t)            (host-folded, float64)  [720, 64]
    P      = eval_matrix @ W_mlp                                   [720, 64]
    v      = eval_matrix @ b_mlp                                   [720]
    q      = P @ sum_t G[t]                                        [720]

    U      = x_row @ G      (x_row = raw x_enc[b, :, e], no normalization!)
    mu     = mean_t(x_row);  sd = sqrt(var_t(x_row) + 1e-5)
    out[t', r] = (P @ U)[t'] + mu_r * (1 - q[t']) + sd_r * v[t']

(the affine weight/bias are ones/zeros per the model setup, and the RevIN
scale cancels through the linear path, leaving the rank-2 mu/sd correction,
which is folded into the second matmul as two extra contraction rows.)

Device kernel per core (2 batches of the 16, data-parallel over batch):
  - x ships as fp8e4m3 (halves the HBM-bound input transfer; the 8 cores
    share HBM stacks so per-core effective bandwidth is only ~150-200 GB/s)
    and feeds the PE directly as a mixed-dtype rhs against bf16 weights
    (fp8 weights lose too much precision on the small 1/T and G columns);
    output is f16
  - t mapped as t = p*6 + a (p = SBUF partition, a = column block)
  - x0 (in 2 chunks, so phase A starts on the first chunk) then x1 lead
    the HWDGE ring; w1+w2 ride the SWDGE ring concurrently; output stores
    go on the two HWDGE rings (SWDGE stores can starve behind DVE 2-port
    ops), with a small final store so the HBM write-receipt tail is short
  - a ~3.5us PE filler train (N=256 junk matmuls) guarantees the HAM
    clock un-throttles before phase A: warm state is sticky across the
    DMA-pacing gaps, but a gappy phase never warms on its own (this was
    worth ~2us of run-to-run variance)
  - phase A per batch: 6 accumulating matmuls [120t x 66] x [120t x 322e]
    -> psum [66, 322] (rows 0,1 = mu via 1/720 cols, rows 2:66 = U^T);
    squares (fp8 elementwise, slow on every engine, so interleaved across
    DVE and ACT idle windows) feed 6 matmuls accumulating E[x^2] into a
    separate psum bank
  - stats: ACT squares mu, DVE folds var = E[x^2] - mu^2 reading psum
    directly, ACT copies psum -> rhs2 bf16 and writes sd = sqrt(var+eps)
    into row 0 (W2's rank-1 rows are ordered [v; 1-q] to match [sd; mu]);
    both batches' stats are issued before the phase-C copies so the sqrt
    chain never queues behind them on ACT
  - phase C: 6 matmuls [66 x 120] x [66 x 322] -> 4 rotating psum banks ->
    f16 copies alternating DVE/ACT -> chunked stores, small final store
    (the last store's ~2us HBM write-receipt is on the critical path)
"""

import numpy as np

BATCH, T, E, N = 16, 720, 321, 64
N_CORES = 8
B_PER_CORE = BATCH // N_CORES   # 2
TT = 120                        # time-tile (partition dim of phase-A matmuls)
NT = T // TT                    # 6
M1 = N + 2                      # 66: two 1/T columns + G columns
EP = E + 1                      # 322: keep moving dim even / 4B-aligned
W1C = NT * M1                   # 396 cols of w1
XB = NT * EP                    # 1932 cols per batch of x
N_FILL = 15                     # PE keep-alive fillers bridging the DMA window
USE_TILE_POS = False             # col-tile the E[x^2] matmuls at array cols 96+
X_FP8 = True                    # ship x as fp8e4m3

_PROGRAM = None


def _fold_weights(A, B_vec, eval_matrix, W_mlp, b_mlp):
    """Host-side weight folding in float64.

    Returns W1 [120, 6, 66] (cols: [1/T, 1/T, G]) and W2 [66, 6, 120]
    (rows: [v, 1-q, P^T]), both bf16, with t mapped as p*6 + a.
    """
    import ml_dtypes

    A64 = np.asarray(A, np.float64)
    Bv = np.asarray(B_vec, np.float64)
    G = np.empty((T, N), np.float64)
    r = Bv.copy()                       # r_k = B^T (A^T)^k
    for k in range(T):
        G[T - 1 - k] = r
        r = r @ A64.T
    P_mat = np.asarray(eval_matrix, np.float64) @ np.asarray(W_mlp, np.float64)
    v = np.asarray(eval_matrix, np.float64) @ np.asarray(b_mlp, np.float64)
    q = P_mat @ G.sum(axis=0)
    W1 = np.concatenate([np.full((T, 2), 1.0 / T), G], axis=1)
    W1 = W1.reshape(TT, NT, M1)                          # [120, 6, 66]
    W2 = np.concatenate([v[None, :], (1.0 - q)[None, :], P_mat.T], axis=0)
    W2 = W2.reshape(M1, TT, NT).transpose(0, 2, 1)       # [66, 6, 120]
    bf16 = ml_dtypes.bfloat16
    return (np.ascontiguousarray(W1).astype(bf16),
            np.ascontiguousarray(W2).astype(bf16))


def _build_program():
    from contextlib import ExitStack

    import concourse.tile as tile
    from concourse import bacc, mybir

    f32 = mybir.dt.float32
    bf16 = mybir.dt.bfloat16
    f16 = mybir.dt.float16
    f8 = mybir.dt.float8e4
    xdt = f8 if X_FP8 else bf16
    nc = bacc.Bacc("TRN2", target_bir_lowering=False, debug=False,
                   num_devices=N_CORES)

    w1 = nc.dram_tensor("w1", [TT, W1C], bf16, kind="ExternalInput")
    xs = nc.dram_tensor("xs", [TT, 2 * XB], xdt, kind="ExternalInput")
    w2 = nc.dram_tensor("w2", [M1, NT * TT], bf16, kind="ExternalInput")
    out = nc.dram_tensor("out", [B_PER_CORE, TT, NT, E], f16, kind="ExternalOutput")

    with tile.TileContext(nc) as tc, ExitStack() as ctx:
        consts = ctx.enter_context(tc.tile_pool(name="consts", bufs=1))
        xpool = ctx.enter_context(tc.tile_pool(name="xpool", bufs=1))
        sqpool = ctx.enter_context(tc.tile_pool(name="sqpool", bufs=1))
        stats = ctx.enter_context(tc.tile_pool(name="stats", bufs=1))
        opool = ctx.enter_context(tc.tile_pool(name="opool", bufs=1))
        psum_a = ctx.enter_context(tc.tile_pool(name="psum_a", bufs=1, space="PSUM"))
        psum_s = ctx.enter_context(tc.tile_pool(name="psum_s", bufs=1, space="PSUM"))
        psum_o = ctx.enter_context(tc.tile_pool(name="psum_o", bufs=1, space="PSUM"))

        # ---- input DMAs first: x0 gates phase A so it leads the HWDGE
        # ring; weights ride SWDGE concurrently
        x_sb = xpool.tile([TT, 2 * XB], xdt, name="x_sb")
        w1_sb = consts.tile([TT, W1C], bf16, name="w1_sb")
        w2_sb = consts.tile([M1, NT * TT], bf16, name="w2_sb")
        HXB = XB // 2
        nc.sync.dma_start(out=x_sb[:, 0:HXB], in_=xs[:, 0:HXB])
        nc.gpsimd.dma_start(out=w1_sb, in_=w1[:])
        nc.sync.dma_start(out=x_sb[:, HXB:XB], in_=xs[:, HXB:XB])
        nc.sync.dma_start(out=x_sb[:, XB:], in_=xs[:, XB:])
        nc.gpsimd.dma_start(out=w2_sb, in_=w2[:])

        # ---- tiny consts + ACT table preload + PE fillers ----
        eps_sb = consts.tile([1, 1], f32)
        wf = consts.tile([128, 272], bf16)
        nc.vector.memset(eps_sb, 1e-5)
        nc.vector.memset(wf, 1.0)
        dsq = consts.tile([1, 1], f32)
        nc.scalar.activation(dsq[:, :], eps_sb[:, :],
                             mybir.ActivationFunctionType.Sqrt,
                             bias=eps_sb[:, :])
        nc.scalar.square(dsq[:, :], eps_sb[:, :])
        pw = psum_o.tile([TT, EP], f32, tag="po_3", name="pw")
        for i in range(N_FILL):
            nc.tensor.matmul(pw[0:16, 0:256], lhsT=wf[:, 0:16],
                             rhs=wf[:, 16:272],
                             start=(i == 0), stop=(i == N_FILL - 1))

        xsq = [sqpool.tile([TT, XB], bf16, name=f"xsq_{b}")
               for b in range(B_PER_CORE)]
        p1s = [psum_a.tile([M1, EP], f32, tag=f"p1_{b}", name=f"p1_{b}")
               for b in range(B_PER_CORE)]
        pss = [psum_s.tile([128, EP], f32, tag=f"ps_{b}", name=f"ps_{b}")
               for b in range(B_PER_CORE)]
        SSR = 96 if USE_TILE_POS else 0   # psum row of the E[x^2] accumulator
        TP = (0, SSR) if USE_TILE_POS else None
        out_sb = opool.tile([TT, B_PER_CORE, NT, E], f16, name="out_sb")

        def phase_a(b):
            # U + mu accumulation (PE)
            xoff = b * XB
            for ti in range(NT):
                nc.tensor.matmul(p1s[b][:, :],
                                 lhsT=w1_sb[:, M1 * ti:M1 * ti + M1],
                                 rhs=x_sb[:, xoff + EP * ti:xoff + EP * (ti + 1)],
                                 start=(ti == 0), stop=(ti == NT - 1))

        def squares(b):
            # fp8 elementwise is slow on every engine; interleave DVE/ACT so
            # neither clogs the stats chain
            xoff = b * XB
            for h in range(2):
                cl, cr = XB // 2 * h, XB // 2 * (h + 1)
                if (b, h) in ((0, 0), (0, 1), (1, 1)):
                    nc.vector.tensor_mul(xsq[b][:, cl:cr],
                                         x_sb[:, xoff + cl:xoff + cr],
                                         x_sb[:, xoff + cl:xoff + cr])
                else:
                    nc.scalar.square(xsq[b][:, cl:cr],
                                     x_sb[:, xoff + cl:xoff + cr])

        def phase_ss(b):
            for ti in range(NT):
                nc.tensor.matmul(pss[b][SSR:SSR + 1, :], lhsT=w1_sb[:, 0:1],
                                 rhs=xsq[b][:, EP * ti:EP * (ti + 1)],
                                 start=(ti == 0), stop=(ti == NT - 1),
                                 tile_position=TP)

        rhs2s, musqs, vars_ = [], [], []

        def stats_pre(b):
            # needs only the U-group stop: the rhs2 bulk copy (ACT), then
            # mu^2 on DVE from the bf16 mu row already in rhs2 — SBUF*SBUF
            # is legal on DVE (psum*psum is not), and this keeps mu^2 off
            # the ACT queue where the scheduler kept baking it ahead of
            # the other batch's sqrt
            musq = stats.tile([1, EP], f32, name=f"musq_{b}")
            rhs2 = stats.tile([M1, EP], bf16, name=f"rhs2_{b}")
            musqs.append(musq)
            rhs2s.append(rhs2)
            nc.scalar.copy(rhs2[:, :], p1s[b][:, :])              # mu + U
            nc.vector.tensor_mul(musq[:, :], rhs2[0:1, :], rhs2[0:1, :])

        def stats_post(b):
            # needs the ss-group stop: var fold + sd
            var = stats.tile([1, EP], f32, name=f"var_{b}")
            vars_.append(var)
            nc.vector.tensor_sub(var[:, :], pss[b][SSR:SSR + 1, :],
                                 musqs[b][:, :])
            nc.scalar.activation(rhs2s[b][0:1, :], var[0:1, :],
                                 mybir.ActivationFunctionType.Sqrt,
                                 bias=eps_sb[0:1, :])             # sd

        def phase_c(b):
            rhs2 = rhs2s[b]
            for a in range(NT):
                po = psum_o.tile([TT, EP], f32, tag=f"po_{a % 4}",
                                 name=f"po_{b}_{a}")
                nc.tensor.matmul(po[:, :], lhsT=w2_sb[:, TT * a:TT * (a + 1)],
                                 rhs=rhs2[:, :], start=True, stop=True)
                if a % 2 == 0:
                    nc.vector.tensor_copy(out_sb[:, b, a, :], po[:, 0:E])
                else:
                    nc.scalar.copy(out_sb[:, b, a, :], po[:, 0:E])
                if b == 0 and a == 2:
                    nc.sync.dma_start(out=out[0][:, 0:3, :],
                                      in_=out_sb[:, 0, 0:3, :])
                if b == 1 and a == 2:
                    nc.sync.dma_start(out=out[1][:, 0:3, :],
                                      in_=out_sb[:, 1, 0:3, :])
                if b == 1 and a == 4:
                    nc.sync.dma_start(out=out[1][:, 3:5, :],
                                      in_=out_sb[:, 1, 3:5, :])
            if b == 0:
                nc.sync.dma_start(out=out[0][:, 3:6, :],
                                  in_=out_sb[:, 0, 3:6, :])
            else:
                nc.scalar.dma_start(out=out[1][:, 5:6, :],
                                    in_=out_sb[:, 1, 5:6, :])

        # pipelined issue order; PE queue: U0, ss0, U1, ss1, C0, C1.
        # stats_pre/post for both batches are issued before phase_c(0) so
        # the sqrt chain never queues behind phase-C copies on ACT
        phase_a(0)
        squares(0)
        phase_ss(0)
        stats_pre(0)
        squares(1)
        stats_post(0)
        phase_a(1)
        stats_pre(1)
        phase_ss(1)
        stats_post(1)
        phase_c(0)
        phase_c(1)

    nc.compile()
    return nc


def _get_program():
    global _PROGRAM
    if _PROGRAM is None:
        _PROGRAM = _build_program()
    return _PROGRAM


def _prepare_inputs(x_enc, A, B_vec, eval_matrix, W_mlp, b_mlp):
    import ml_dtypes

    bf16 = ml_dtypes.bfloat16
    xdt = ml_dtypes.float8_e4m3 if X_FP8 else bf16
    x = np.asarray(x_enc, np.float32)
    xp = np.zeros((BATCH, T, EP), np.float32)
    xp[:, :, :E] = x
    # t = p*6 + a layout: [B, 120, 6*322]
    xr = xp.reshape(BATCH, TT, XB).astype(xdt)
    W1, W2 = _fold_weights(A, B_vec, eval_matrix, W_mlp, b_mlp)
    w1_flat = np.ascontiguousarray(W1.reshape(TT, W1C))  # [120, 396]
    w2_flat = np.ascontiguousarray(W2.reshape(M1, NT * TT))
    ins = []
    for k in range(N_CORES):
        b0, b1 = k * B_PER_CORE, k * B_PER_CORE + 1
        xcat = np.concatenate([xr[b0], xr[b1]], axis=1)
        ins.append({"w1": w1_flat, "xs": np.ascontiguousarray(xcat),
                    "w2": w2_flat})
    return ins


def kernel(x_enc, A, B_vec, eval_matrix, W_mlp, b_mlp, affine_weight, affine_bias):
    from concourse.bass_utils import run_bass_kernel_spmd

    nc = _get_program()
    in_maps = _prepare_inputs(x_enc, A, B_vec, eval_matrix, W_mlp, b_mlp)
    res = run_bass_kernel_spmd(nc, in_maps, core_ids=list(range(N_CORES)))
    outs = [np.asarray(res.results[k]["out"]) for k in range(N_CORES)]
    full = np.concatenate(outs, axis=0)            # [16, 120, 6, 321] f16
    full = full.reshape(BATCH, T, E).astype(np.float32)
    return full


# revision 33
# speedup vs baseline: 1.0115x; 1.0115x over previous
"""Trainium2 Bass kernel for nn_Decomp_Forecast (HiPPO-LegS decomposition forecaster).

Math: the reference runs a 720-step linear scan c_t = c_{t-1} @ A^T + f_t * B
and only uses the final state, so the whole model collapses (exactly, by
associativity) to two chained matmuls around the instance-norm statistics:

    G[t]   = B^T (A^T)^(T-1-t)            (host-folded, float64)  [720, 64]
    P      = eval_matrix @ W_mlp                                   [720, 64]
    v      = eval_matrix @ b_mlp                                   [720]
    q      = P @ sum_t G[t]                                        [720]

    U      = x_row @ G      (x_row = raw x_enc[b, :, e], no normalization!)
    mu     = mean_t(x_row);  sd = sqrt(var_t(x_row) + 1e-5)
    out[t', r] = (P @ U)[t'] + mu_r * (1 - q[t']) + sd_r * v[t']

(the affine weight/bias are ones/zeros per the model setup, and the RevIN
scale cancels through the linear path, leaving the rank-2 mu/sd correction,
which is folded into the second matmul as two extra contraction rows.)

Device kernel per core (2 batches of the 16, data-parallel over batch):
  - x ships as fp8e4m3 (halves the HBM-bound input transfer; the 8 cores
    share HBM stacks so per-core effective bandwidth is only ~150-200 GB/s)
    and feeds the PE directly as a mixed-dtype rhs against bf16 weights
    (fp8 weights lose too much precision on the small 1/T and G columns);
    output is f16
  - t mapped as t = p*6 + a (p = SBUF partition, a = column block)
  - x0 (in 2 chunks, so phase A starts on the first chunk) then x1 lead
    the HWDGE ring; w1+w2 ride the SWDGE ring concurrently; output stores
    go on the two HWDGE rings (SWDGE stores can starve behind DVE 2-port
    ops), with a small final store so the HBM write-receipt tail is short
  - a ~3.5us PE filler train (N=256 junk matmuls) guarantees the HAM
    clock un-throttles before phase A: warm state is sticky across the
    DMA-pacing gaps, but a gappy phase never warms on its own (this was
    worth ~2us of run-to-run variance)
  - phase A per batch: 6 accumulating matmuls [120t x 66] x [120t x 322e]
    -> psum [66, 322] (rows 0,1 = mu via 1/720 cols, rows 2:66 = U^T);
    squares (fp8 elementwise, slow on every engine, so interleaved across
    DVE and ACT idle windows) feed 6 matmuls accumulating E[x^2] into a
    separate psum bank
  - stats: ACT squares mu, DVE folds var = E[x^2] - mu^2 reading psum
    directly, ACT copies psum -> rhs2 bf16 and writes sd = sqrt(var+eps)
    into row 0 (W2's rank-1 rows are ordered [v; 1-q] to match [sd; mu]);
    both batches' stats are issued before the phase-C copies so the sqrt
    chain never queues behind them on ACT
  - phase C: 6 matmuls [66 x 120] x [66 x 322] -> 4 rotating psum banks ->
    f16 copies alternating DVE/ACT -> chunked stores, small final store
    (the last store's ~2us HBM write-receipt is on the critical path)
"""

import numpy as np

BATCH, T, E, N = 16, 720, 321, 64
N_CORES = 8
B_PER_CORE = BATCH // N_CORES   # 2
TT = 120                        # time-tile (partition dim of phase-A matmuls)
NT = T // TT                    # 6
M1 = N + 2                      # 66: two 1/T columns + G columns
EP = E + 1                      # 322: keep moving dim even / 4B-aligned
W1C = NT * M1                   # 396 cols of w1
XB = NT * EP                    # 1932 cols per batch of x
N_FILL = 15                     # PE keep-alive fillers bridging the DMA window
USE_TILE_POS = False             # col-tile the E[x^2] matmuls at array cols 96+
X_FP8 = True                    # ship x as fp8e4m3

_PROGRAM = None


def _fold_weights(A, B_vec, eval_matrix, W_mlp, b_mlp):
    """Host-side weight folding in float64.

    Returns W1 [120, 6, 66] (cols: [1/T, 1/T, G]) and W2 [66, 6, 120]
    (rows: [v, 1-q, P^T]), both bf16, with t mapped as p*6 + a.
    """
    import ml_dtypes

    A64 = np.asarray(A, np.float64)
    Bv = np.asarray(B_vec, np.float64)
    G = np.empty((T, N), np.float64)
    r = Bv.copy()                       # r_k = B^T (A^T)^k
    for k in range(T):
        G[T - 1 - k] = r
        r = r @ A64.T
    P_mat = np.asarray(eval_matrix, np.float64) @ np.asarray(W_mlp, np.float64)
    v = np.asarray(eval_matrix, np.float64) @ np.asarray(b_mlp, np.float64)
    q = P_mat @ G.sum(axis=0)
    W1 = np.concatenate([np.full((T, 2), 1.0 / T), G], axis=1)
    W1 = W1.reshape(TT, NT, M1)                          # [120, 6, 66]
    W2 = np.concatenate([v[None, :], (1.0 - q)[None, :], P_mat.T], axis=0)
    W2 = W2.reshape(M1, TT, NT).transpose(0, 2, 1)       # [66, 6, 120]
    bf16 = ml_dtypes.bfloat16
    return (np.ascontiguousarray(W1).astype(bf16),
            np.ascontiguousarray(W2).astype(bf16))


def _build_program():
    from contextlib import ExitStack

    import concourse.tile as tile
    from concourse import bacc, mybir

    f32 = mybir.dt.float32
    bf16 = mybir.dt.bfloat16
    f16 = mybir.dt.float16
    f8 = mybir.dt.float8e4
    xdt = f8 if X_FP8 else bf16
    nc = bacc.Bacc("TRN2", target_bir_lowering=False, debug=False,
                   num_devices=N_CORES)

    w1 = nc.dram_tensor("w1", [TT, W1C], bf16, kind="ExternalInput")
    xs = nc.dram_tensor("xs", [TT, 2 * XB], xdt, kind="ExternalInput")
    w2 = nc.dram_tensor("w2", [M1, NT * TT], bf16, kind="ExternalInput")
    out = nc.dram_tensor("out", [B_PER_CORE, TT, NT, E], f16, kind="ExternalOutput")

    with tile.TileContext(nc) as tc, ExitStack() as ctx:
        consts = ctx.enter_context(tc.tile_pool(name="consts", bufs=1))
        xpool = ctx.enter_context(tc.tile_pool(name="xpool", bufs=1))
        sqpool = ctx.enter_context(tc.tile_pool(name="sqpool", bufs=1))
        stats = ctx.enter_context(tc.tile_pool(name="stats", bufs=1))
        opool = ctx.enter_context(tc.tile_pool(name="opool", bufs=1))
        psum_a = ctx.enter_context(tc.tile_pool(name="psum_a", bufs=1, space="PSUM"))
        psum_s = ctx.enter_context(tc.tile_pool(name="psum_s", bufs=1, space="PSUM"))
        psum_o = ctx.enter_context(tc.tile_pool(name="psum_o", bufs=1, space="PSUM"))

        # ---- input DMAs first: x0 gates phase A so it leads the HWDGE
        # ring; weights ride SWDGE concurrently
        x_sb = xpool.tile([TT, 2 * XB], xdt, name="x_sb")
        w1_sb = consts.tile([TT, W1C], bf16, name="w1_sb")
        w2_sb = consts.tile([M1, NT * TT], bf16, name="w2_sb")
        HXB = XB // 2
        nc.sync.dma_start(out=x_sb[:, 0:HXB], in_=xs[:, 0:HXB])
        nc.gpsimd.dma_start(out=w1_sb, in_=w1[:])
        nc.sync.dma_start(out=x_sb[:, HXB:XB], in_=xs[:, HXB:XB])
        nc.sync.dma_start(out=x_sb[:, XB:], in_=xs[:, XB:])
        nc.gpsimd.dma_start(out=w2_sb, in_=w2[:])

        # ---- tiny consts + ACT table preload + PE fillers ----
        eps_sb = consts.tile([1, 1], f32)
        wf = consts.tile([128, 272], bf16)
        nc.vector.memset(eps_sb, 1e-5)
        nc.vector.memset(wf, 1.0)
        dsq = consts.tile([1, 1], f32)
        nc.scalar.activation(dsq[:, :], eps_sb[:, :],
                             mybir.ActivationFunctionType.Sqrt,
                             bias=eps_sb[:, :])
        nc.scalar.square(dsq[:, :], eps_sb[:, :])
        pw = psum_o.tile([TT, EP], f32, tag="po_3", name="pw")
        for i in range(N_FILL):
            nc.tensor.matmul(pw[0:16, 0:256], lhsT=wf[:, 0:16],
                             rhs=wf[:, 16:272],
                             start=(i == 0), stop=(i == N_FILL - 1))

        xsq = [sqpool.tile([TT, XB], bf16, name=f"xsq_{b}")
               for b in range(B_PER_CORE)]
        p1s = [psum_a.tile([M1, EP], f32, tag=f"p1_{b}", name=f"p1_{b}")
               for b in range(B_PER_CORE)]
        pss = [psum_s.tile([128, EP], f32, tag=f"ps_{b}", name=f"ps_{b}")
               for b in range(B_PER_CORE)]
        SSR = 96 if USE_TILE_POS else 0   # psum row of the E[x^2] accumulator
        TP = (0, SSR) if USE_TILE_POS else None
        out_sb = opool.tile([TT, B_PER_CORE, NT, E], f16, name="out_sb")

        def phase_a(b):
            # U + mu accumulation (PE)
            xoff = b * XB
            for ti in range(NT):
                nc.tensor.matmul(p1s[b][:, :],
                                 lhsT=w1_sb[:, M1 * ti:M1 * ti + M1],
                                 rhs=x_sb[:, xoff + EP * ti:xoff + EP * (ti + 1)],
                                 start=(ti == 0), stop=(ti == NT - 1))

        def squares(b):
            # fp8 elementwise is slow on every engine; interleave DVE/ACT so
            # neither clogs the stats chain
            xoff = b * XB
            for h in range(2):
                cl, cr = XB // 2 * h, XB // 2 * (h + 1)
                if (b, h) in ((0, 0), (0, 1), (1, 1)):
                    nc.vector.tensor_mul(xsq[b][:, cl:cr],
                                         x_sb[:, xoff + cl:xoff + cr],
                                         x_sb[:, xoff + cl:xoff + cr])
                else:
                    nc.scalar.square(xsq[b][:, cl:cr],
                                     x_sb[:, xoff + cl:xoff + cr])

        def phase_ss(b):
            for ti in range(NT):
                nc.tensor.matmul(pss[b][SSR:SSR + 1, :], lhsT=w1_sb[:, 0:1],
                                 rhs=xsq[b][:, EP * ti:EP * (ti + 1)],
                                 start=(ti == 0), stop=(ti == NT - 1),
                                 tile_position=TP)

        rhs2s, musqs, vars_ = [], [], []

        def stats_pre(b):
            # needs only the U-group stop: mu^2 and the rhs2 bulk copy.
            # b0's mu^2 stays on ACT (fastest path to sub0); b1's moves to
            # DVE, squaring the bf16 mu in rhs2 row 0 (psum rows 0 and 1
            # are both mu) — this removes the one ACT op the scheduler
            # kept baking ahead of sqrt0, a measured ~0.9us stall on C0
            musq = stats.tile([1, EP], f32, name=f"musq_{b}")
            rhs2 = stats.tile([M1, EP], bf16, name=f"rhs2_{b}")
            musqs.append(musq)
            rhs2s.append(rhs2)
            nc.scalar.copy(rhs2[:, :], p1s[b][:, :])              # mu + U
            if b == 0:
                nc.scalar.square(musq[:, :], p1s[b][0:1, :])
            else:
                nc.vector.tensor_mul(musq[:, :], rhs2[0:1, :], rhs2[0:1, :])

        def stats_post(b):
            # needs the ss-group stop: var fold + sd
            var = stats.tile([1, EP], f32, name=f"var_{b}")
            vars_.append(var)
            nc.vector.tensor_sub(var[:, :], pss[b][SSR:SSR + 1, :],
                                 musqs[b][:, :])
            nc.scalar.activation(rhs2s[b][0:1, :], var[0:1, :],
                                 mybir.ActivationFunctionType.Sqrt,
                                 bias=eps_sb[0:1, :])             # sd

        def phase_c(b):
            rhs2 = rhs2s[b]
            for a in range(NT):
                po = psum_o.tile([TT, EP], f32, tag=f"po_{a % 4}",
                                 name=f"po_{b}_{a}")
                nc.tensor.matmul(po[:, :], lhsT=w2_sb[:, TT * a:TT * (a + 1)],
                                 rhs=rhs2[:, :], start=True, stop=True)
                if a % 2 == 0:
                    nc.vector.tensor_copy(out_sb[:, b, a, :], po[:, 0:E])
                else:
                    nc.scalar.copy(out_sb[:, b, a, :], po[:, 0:E])
                if b == 0 and a == 2:
                    nc.sync.dma_start(out=out[0][:, 0:3, :],
                                      in_=out_sb[:, 0, 0:3, :])
                if b == 1 and a == 2:
                    nc.sync.dma_start(out=out[1][:, 0:3, :],
                                      in_=out_sb[:, 1, 0:3, :])
                if b == 1 and a == 4:
                    nc.sync.dma_start(out=out[1][:, 3:5, :],
                                      in_=out_sb[:, 1, 3:5, :])
            if b == 0:
                nc.sync.dma_start(out=out[0][:, 3:6, :],
                                  in_=out_sb[:, 0, 3:6, :])
            else:
                nc.scalar.dma_start(out=out[1][:, 5:6, :],
                                    in_=out_sb[:, 1, 5:6, :])

        # pipelined issue order; PE queue: U0, ss0, U1, ss1, C0, C1.
        # stats_pre/post for both batches are issued before phase_c(0) so
        # the sqrt chain never queues behind phase-C copies on ACT
        phase_a(0)
        squares(0)
        phase_ss(0)
        stats_pre(0)
        squares(1)
        stats_post(0)
        phase_a(1)
        stats_pre(1)
        phase_ss(1)
        stats_post(1)
        phase_c(0)
        phase_c(1)

    nc.compile()
    return nc


def _get_program():
    global _PROGRAM
    if _PROGRAM is None:
        _PROGRAM = _build_program()
    return _PROGRAM


def _prepare_inputs(x_enc, A, B_vec, eval_matrix, W_mlp, b_mlp):
    import ml_dtypes

    bf16 = ml_dtypes.bfloat16
    xdt = ml_dtypes.float8_e4m3 if X_FP8 else bf16
    x = np.asarray(x_enc, np.float32)
    xp = np.zeros((BATCH, T, EP), np.float32)
    xp[:, :, :E] = x
    # t = p*6 + a layout: [B, 120, 6*322]
    xr = xp.reshape(BATCH, TT, XB).astype(xdt)
    W1, W2 = _fold_weights(A, B_vec, eval_matrix, W_mlp, b_mlp)
    w1_flat = np.ascontiguousarray(W1.reshape(TT, W1C))  # [120, 396]
    w2_flat = np.ascontiguousarray(W2.reshape(M1, NT * TT))
    ins = []
    for k in range(N_CORES):
        b0, b1 = k * B_PER_CORE, k * B_PER_CORE + 1
        xcat = np.concatenate([xr[b0], xr[b1]], axis=1)
        ins.append({"w1": w1_flat, "xs": np.ascontiguousarray(xcat),
                    "w2": w2_flat})
    return ins


def kernel(x_enc, A, B_vec, eval_matrix, W_mlp, b_mlp, affine_weight, affine_bias):
    from concourse.bass_utils import run_bass_kernel_spmd

    nc = _get_program()
    in_maps = _prepare_inputs(x_enc, A, B_vec, eval_matrix, W_mlp, b_mlp)
    res = run_bass_kernel_spmd(nc, in_maps, core_ids=list(range(N_CORES)))
    outs = [np.asarray(res.results[k]["out"]) for k in range(N_CORES)]
    full = np.concatenate(outs, axis=0)            # [16, 120, 6, 321] f16
    full = full.reshape(BATCH, T, E).astype(np.float32)
    return full


# revision 35
# speedup vs baseline: 1.0126x; 1.0011x over previous
"""Trainium2 Bass kernel for nn_Decomp_Forecast (HiPPO-LegS decomposition forecaster).

Math: the reference runs a 720-step linear scan c_t = c_{t-1} @ A^T + f_t * B
and only uses the final state, so the whole model collapses (exactly, by
associativity) to two chained matmuls around the instance-norm statistics:

    G[t]   = B^T (A^T)^(T-1-t)            (host-folded, float64)  [720, 64]
    P      = eval_matrix @ W_mlp                                   [720, 64]
    v      = eval_matrix @ b_mlp                                   [720]
    q      = P @ sum_t G[t]                                        [720]

    U      = x_row @ G      (x_row = raw x_enc[b, :, e], no normalization!)
    mu     = mean_t(x_row);  sd = sqrt(var_t(x_row) + 1e-5)
    out[t', r] = (P @ U)[t'] + mu_r * (1 - q[t']) + sd_r * v[t']

(the affine weight/bias are ones/zeros per the model setup, and the RevIN
scale cancels through the linear path, leaving the rank-2 mu/sd correction,
which is folded into the second matmul as two extra contraction rows.)

Device kernel per core (2 batches of the 16, data-parallel over batch):
  - x ships as fp8e4m3 (halves the HBM-bound input transfer; the 8 cores
    share HBM stacks so per-core effective bandwidth is only ~150-200 GB/s)
    and feeds the PE directly as a mixed-dtype rhs against bf16 weights
    (fp8 weights lose too much precision on the small 1/T and G columns);
    output is f16
  - t mapped as t = p*6 + a (p = SBUF partition, a = column block)
  - x0 (in 2 chunks, so phase A starts on the first chunk) then x1 lead
    the HWDGE ring; w1+w2 ride the SWDGE ring concurrently; output stores
    go on the two HWDGE rings (SWDGE stores can starve behind DVE 2-port
    ops), with a small final store so the HBM write-receipt tail is short
  - a ~3.5us PE filler train (N=256 junk matmuls) guarantees the HAM
    clock un-throttles before phase A: warm state is sticky across the
    DMA-pacing gaps, but a gappy phase never warms on its own (this was
    worth ~2us of run-to-run variance)
  - phase A per batch: 6 accumulating matmuls [120t x 66] x [120t x 322e]
    -> psum [66, 322] (rows 0,1 = mu via 1/720 cols, rows 2:66 = U^T);
    squares (fp8 elementwise, slow on every engine, so interleaved across
    DVE and ACT idle windows) feed 6 matmuls accumulating E[x^2] into a
    separate psum bank
  - stats: ACT squares mu, DVE folds var = E[x^2] - mu^2 reading psum
    directly, ACT copies psum -> rhs2 bf16 and writes sd = sqrt(var+eps)
    into row 0 (W2's rank-1 rows are ordered [v; 1-q] to match [sd; mu]);
    both batches' stats are issued before the phase-C copies so the sqrt
    chain never queues behind them on ACT
  - phase C: 6 matmuls [66 x 120] x [66 x 322] -> 4 rotating psum banks ->
    f16 copies alternating DVE/ACT -> chunked stores, small final store
    (the last store's ~2us HBM write-receipt is on the critical path)
"""

import numpy as np

BATCH, T, E, N = 16, 720, 321, 64
N_CORES = 8
B_PER_CORE = BATCH // N_CORES   # 2
TT = 120                        # time-tile (partition dim of phase-A matmuls)
NT = T // TT                    # 6
M1 = N + 2                      # 66: two 1/T columns + G columns
EP = E + 1                      # 322: keep moving dim even / 4B-aligned
W1C = NT * M1                   # 396 cols of w1
XB = NT * EP                    # 1932 cols per batch of x
N_FILL = 15                     # PE keep-alive fillers bridging the DMA window
USE_TILE_POS = False             # col-tile the E[x^2] matmuls at array cols 96+
X_FP8 = True                    # ship x as fp8e4m3

_PROGRAM = None


def _fold_weights(A, B_vec, eval_matrix, W_mlp, b_mlp):
    """Host-side weight folding in float64.

    Returns W1 [120, 6, 66] (cols: [1/T, 1/T, G]) and W2 [66, 6, 120]
    (rows: [v, 1-q, P^T]), both bf16, with t mapped as p*6 + a.
    """
    import ml_dtypes

    A64 = np.asarray(A, np.float64)
    Bv = np.asarray(B_vec, np.float64)
    G = np.empty((T, N), np.float64)
    r = Bv.copy()                       # r_k = B^T (A^T)^k
    for k in range(T):
        G[T - 1 - k] = r
        r = r @ A64.T
    P_mat = np.asarray(eval_matrix, np.float64) @ np.asarray(W_mlp, np.float64)
    v = np.asarray(eval_matrix, np.float64) @ np.asarray(b_mlp, np.float64)
    q = P_mat @ G.sum(axis=0)
    W1 = np.concatenate([np.full((T, 2), 1.0 / T), G], axis=1)
    W1 = W1.reshape(TT, NT, M1)                          # [120, 6, 66]
    W2 = np.concatenate([v[None, :], (1.0 - q)[None, :], P_mat.T], axis=0)
    W2 = W2.reshape(M1, TT, NT).transpose(0, 2, 1)       # [66, 6, 120]
    bf16 = ml_dtypes.bfloat16
    return (np.ascontiguousarray(W1).astype(bf16),
            np.ascontiguousarray(W2).astype(bf16))


def _build_program():
    from contextlib import ExitStack

    import concourse.tile as tile
    from concourse import bacc, mybir

    f32 = mybir.dt.float32
    bf16 = mybir.dt.bfloat16
    f16 = mybir.dt.float16
    f8 = mybir.dt.float8e4
    xdt = f8 if X_FP8 else bf16
    nc = bacc.Bacc("TRN2", target_bir_lowering=False, debug=False,
                   num_devices=N_CORES)

    w1 = nc.dram_tensor("w1", [TT, W1C], bf16, kind="ExternalInput")
    xs = nc.dram_tensor("xs", [TT, 2 * XB], xdt, kind="ExternalInput")
    w2 = nc.dram_tensor("w2", [M1, NT * TT], bf16, kind="ExternalInput")
    out = nc.dram_tensor("out", [B_PER_CORE, TT, NT, E], f16, kind="ExternalOutput")

    with tile.TileContext(nc) as tc, ExitStack() as ctx:
        consts = ctx.enter_context(tc.tile_pool(name="consts", bufs=1))
        xpool = ctx.enter_context(tc.tile_pool(name="xpool", bufs=1))
        sqpool = ctx.enter_context(tc.tile_pool(name="sqpool", bufs=1))
        stats = ctx.enter_context(tc.tile_pool(name="stats", bufs=1))
        opool = ctx.enter_context(tc.tile_pool(name="opool", bufs=1))
        psum_a = ctx.enter_context(tc.tile_pool(name="psum_a", bufs=1, space="PSUM"))
        psum_s = ctx.enter_context(tc.tile_pool(name="psum_s", bufs=1, space="PSUM"))
        psum_o = ctx.enter_context(tc.tile_pool(name="psum_o", bufs=1, space="PSUM"))

        # ---- input DMAs first: x0 gates phase A so it leads the HWDGE
        # ring; weights ride SWDGE concurrently
        x_sb = xpool.tile([TT, 2 * XB], xdt, name="x_sb")
        w1_sb = consts.tile([TT, W1C], bf16, name="w1_sb")
        w2_sb = consts.tile([M1, NT * TT], bf16, name="w2_sb")
        HXB = XB // 2
        nc.sync.dma_start(out=x_sb[:, 0:HXB], in_=xs[:, 0:HXB])
        nc.gpsimd.dma_start(out=w1_sb, in_=w1[:])
        nc.sync.dma_start(out=x_sb[:, HXB:XB], in_=xs[:, HXB:XB])
        nc.sync.dma_start(out=x_sb[:, XB:XB + HXB], in_=xs[:, XB:XB + HXB])
        nc.sync.dma_start(out=x_sb[:, XB + HXB:], in_=xs[:, XB + HXB:])
        nc.gpsimd.dma_start(out=w2_sb, in_=w2[:])

        # ---- tiny consts + ACT table preload + PE fillers ----
        eps_sb = consts.tile([1, 1], f32)
        wf = consts.tile([128, 272], bf16)
        nc.vector.memset(eps_sb, 1e-5)
        nc.vector.memset(wf, 1.0)
        dsq = consts.tile([1, 1], f32)
        nc.scalar.activation(dsq[:, :], eps_sb[:, :],
                             mybir.ActivationFunctionType.Sqrt,
                             bias=eps_sb[:, :])
        nc.scalar.square(dsq[:, :], eps_sb[:, :])
        pw = psum_o.tile([TT, EP], f32, tag="po_3", name="pw")
        for i in range(N_FILL):
            nc.tensor.matmul(pw[0:16, 0:256], lhsT=wf[:, 0:16],
                             rhs=wf[:, 16:272],
                             start=(i == 0), stop=(i == N_FILL - 1))

        xsq = [sqpool.tile([TT, XB], bf16, name=f"xsq_{b}")
               for b in range(B_PER_CORE)]
        p1s = [psum_a.tile([M1, EP], f32, tag=f"p1_{b}", name=f"p1_{b}")
               for b in range(B_PER_CORE)]
        pss = [psum_s.tile([128, EP], f32, tag=f"ps_{b}", name=f"ps_{b}")
               for b in range(B_PER_CORE)]
        SSR = 96 if USE_TILE_POS else 0   # psum row of the E[x^2] accumulator
        TP = (0, SSR) if USE_TILE_POS else None
        out_sb = opool.tile([TT, B_PER_CORE, NT, E], f16, name="out_sb")

        def phase_a(b):
            # U + mu accumulation (PE)
            xoff = b * XB
            for ti in range(NT):
                nc.tensor.matmul(p1s[b][:, :],
                                 lhsT=w1_sb[:, M1 * ti:M1 * ti + M1],
                                 rhs=x_sb[:, xoff + EP * ti:xoff + EP * (ti + 1)],
                                 start=(ti == 0), stop=(ti == NT - 1))

        def squares(b):
            # fp8 elementwise is slow on every engine; interleave DVE/ACT so
            # neither clogs the stats chain
            xoff = b * XB
            for h in range(2):
                cl, cr = XB // 2 * h, XB // 2 * (h + 1)
                if (b, h) in ((0, 0), (0, 1), (1, 1)):
                    nc.vector.tensor_mul(xsq[b][:, cl:cr],
                                         x_sb[:, xoff + cl:xoff + cr],
                                         x_sb[:, xoff + cl:xoff + cr])
                else:
                    nc.scalar.square(xsq[b][:, cl:cr],
                                     x_sb[:, xoff + cl:xoff + cr])

        def phase_ss(b):
            for ti in range(NT):
                nc.tensor.matmul(pss[b][SSR:SSR + 1, :], lhsT=w1_sb[:, 0:1],
                                 rhs=xsq[b][:, EP * ti:EP * (ti + 1)],
                                 start=(ti == 0), stop=(ti == NT - 1),
                                 tile_position=TP)

        rhs2s, musqs, vars_ = [], [], []

        def stats_pre(b):
            # needs only the U-group stop: mu^2 and the rhs2 bulk copy
            musq = stats.tile([1, EP], f32, name=f"musq_{b}")
            rhs2 = stats.tile([M1, EP], bf16, name=f"rhs2_{b}")
            musqs.append(musq)
            rhs2s.append(rhs2)
            nc.scalar.square(musq[:, :], p1s[b][0:1, :])
            nc.scalar.copy(rhs2[:, :], p1s[b][:, :])              # mu + U

        def stats_post(b):
            # needs the ss-group stop: var fold + sd
            var = stats.tile([1, EP], f32, name=f"var_{b}")
            vars_.append(var)
            nc.vector.tensor_sub(var[:, :], pss[b][SSR:SSR + 1, :],
                                 musqs[b][:, :])
            nc.scalar.activation(rhs2s[b][0:1, :], var[0:1, :],
                                 mybir.ActivationFunctionType.Sqrt,
                                 bias=eps_sb[0:1, :])             # sd

        def phase_c(b):
            rhs2 = rhs2s[b]
            for a in range(NT):
                po = psum_o.tile([TT, EP], f32, tag=f"po_{a % 4}",
                                 name=f"po_{b}_{a}")
                nc.tensor.matmul(po[:, :], lhsT=w2_sb[:, TT * a:TT * (a + 1)],
                                 rhs=rhs2[:, :], start=True, stop=True)
                if a % 2 == 0:
                    nc.vector.tensor_copy(out_sb[:, b, a, :], po[:, 0:E])
                else:
                    nc.scalar.copy(out_sb[:, b, a, :], po[:, 0:E])
                if b == 0 and a == 2:
                    nc.sync.dma_start(out=out[0][:, 0:3, :],
                                      in_=out_sb[:, 0, 0:3, :])
                if b == 1 and a == 2:
                    nc.sync.dma_start(out=out[1][:, 0:3, :],
                                      in_=out_sb[:, 1, 0:3, :])
                if b == 1 and a == 4:
                    nc.sync.dma_start(out=out[1][:, 3:5, :],
                                      in_=out_sb[:, 1, 3:5, :])
            if b == 0:
                nc.sync.dma_start(out=out[0][:, 3:6, :],
                                  in_=out_sb[:, 0, 3:6, :])
            else:
                nc.scalar.dma_start(out=out[1][:, 5:6, :],
                                    in_=out_sb[:, 1, 5:6, :])

        # pipelined issue order; PE queue: U0, ss0, U1, ss1, C0, C1.
        # stats_pre/post for both batches are issued before phase_c(0) so
        # the sqrt chain never queues behind phase-C copies on ACT
        phase_a(0)
        squares(0)
        phase_ss(0)
        stats_pre(0)
        squares(1)
        stats_post(0)
        phase_a(1)
        stats_pre(1)
        phase_ss(1)
        stats_post(1)
        phase_c(0)
        phase_c(1)

    nc.compile()
    return nc


def _get_program():
    global _PROGRAM
    if _PROGRAM is None:
        _PROGRAM = _build_program()
    return _PROGRAM


def _prepare_inputs(x_enc, A, B_vec, eval_matrix, W_mlp, b_mlp):
    import ml_dtypes

    bf16 = ml_dtypes.bfloat16
    xdt = ml_dtypes.float8_e4m3 if X_FP8 else bf16
    x = np.asarray(x_enc, np.float32)
    xp = np.zeros((BATCH, T, EP), np.float32)
    xp[:, :, :E] = x
    # t = p*6 + a layout: [B, 120, 6*322]
    xr = xp.reshape(BATCH, TT, XB).astype(xdt)
    W1, W2 = _fold_weights(A, B_vec, eval_matrix, W_mlp, b_mlp)
    w1_flat = np.ascontiguousarray(W1.reshape(TT, W1C))  # [120, 396]
    w2_flat = np.ascontiguousarray(W2.reshape(M1, NT * TT))
    ins = []
    for k in range(N_CORES):
        b0, b1 = k * B_PER_CORE, k * B_PER_CORE + 1
        xcat = np.concatenate([xr[b0], xr[b1]], axis=1)
        ins.append({"w1": w1_flat, "xs": np.ascontiguousarray(xcat),
                    "w2": w2_flat})
    return ins


def kernel(x_enc, A, B_vec, eval_matrix, W_mlp, b_mlp, affine_weight, affine_bias):
    from concourse.bass_utils import run_bass_kernel_spmd

    nc = _get_program()
    in_maps = _prepare_inputs(x_enc, A, B_vec, eval_matrix, W_mlp, b_mlp)
    res = run_bass_kernel_spmd(nc, in_maps, core_ids=list(range(N_CORES)))
    outs = [np.asarray(res.results[k]["out"]) for k in range(N_CORES)]
    full = np.concatenate(outs, axis=0)            # [16, 120, 6, 321] f16
    full = full.reshape(BATCH, T, E).astype(np.float32)
    return full


# revision 37
# speedup vs baseline: 1.0210x; 1.0083x over previous
"""Trainium2 Bass kernel for nn_Decomp_Forecast (HiPPO-LegS decomposition forecaster).

Math: the reference runs a 720-step linear scan c_t = c_{t-1} @ A^T + f_t * B
and only uses the final state, so the whole model collapses (exactly, by
associativity) to two chained matmuls around the instance-norm statistics:

    G[t]   = B^T (A^T)^(T-1-t)            (host-folded, float64)  [720, 64]
    P      = eval_matrix @ W_mlp                                   [720, 64]
    v      = eval_matrix @ b_mlp                                   [720]
    q      = P @ sum_t G[t]                                        [720]

    U      = x_row @ G      (x_row = raw x_enc[b, :, e], no normalization!)
    mu     = mean_t(x_row);  sd = sqrt(var_t(x_row) + 1e-5)
    out[t', r] = (P @ U)[t'] + mu_r * (1 - q[t']) + sd_r * v[t']

(the affine weight/bias are ones/zeros per the model setup, and the RevIN
scale cancels through the linear path, leaving the rank-2 mu/sd correction,
which is folded into the second matmul as two extra contraction rows.)

Device kernel per core (2 batches of the 16, data-parallel over batch):
  - x ships as fp8e4m3 (halves the HBM-bound input transfer; the 8 cores
    share HBM stacks so per-core effective bandwidth is only ~150-200 GB/s)
    and feeds the PE directly as a mixed-dtype rhs against bf16 weights
    (fp8 weights lose too much precision on the small 1/T and G columns);
    output is f16
  - t mapped as t = p*6 + a (p = SBUF partition, a = column block)
  - x0 (in 2 chunks, so phase A starts on the first chunk) then x1 lead
    the HWDGE ring; w1+w2 ride the SWDGE ring concurrently; output stores
    go on the two HWDGE rings (SWDGE stores can starve behind DVE 2-port
    ops), with a small final store so the HBM write-receipt tail is short
  - a ~3.5us PE filler train (N=256 junk matmuls) guarantees the HAM
    clock un-throttles before phase A: warm state is sticky across the
    DMA-pacing gaps, but a gappy phase never warms on its own (this was
    worth ~2us of run-to-run variance)
  - phase A per batch: 6 accumulating matmuls [120t x 66] x [120t x 322e]
    -> psum [66, 322] (rows 0,1 = mu via 1/720 cols, rows 2:66 = U^T);
    squares (fp8 elementwise, slow on every engine, so interleaved across
    DVE and ACT idle windows) feed 6 matmuls accumulating E[x^2] into a
    separate psum bank
  - stats: ACT squares mu, DVE folds var = E[x^2] - mu^2 reading psum
    directly, ACT copies psum -> rhs2 bf16 and writes sd = sqrt(var+eps)
    into row 0 (W2's rank-1 rows are ordered [v; 1-q] to match [sd; mu]);
    both batches' stats are issued before the phase-C copies so the sqrt
    chain never queues behind them on ACT
  - phase C: 6 matmuls [66 x 120] x [66 x 322] -> 4 rotating psum banks ->
    f16 copies alternating DVE/ACT -> chunked stores, small final store
    (the last store's ~2us HBM write-receipt is on the critical path)
"""

import numpy as np

BATCH, T, E, N = 16, 720, 321, 64
N_CORES = 8
B_PER_CORE = BATCH // N_CORES   # 2
TT = 120                        # time-tile (partition dim of phase-A matmuls)
NT = T // TT                    # 6
M1 = N + 2                      # 66: two 1/T columns + G columns
EP = E + 1                      # 322: keep moving dim even / 4B-aligned
W1C = NT * M1                   # 396 cols of w1
XB = NT * EP                    # 1932 cols per batch of x
N_FILL = 15                     # PE keep-alive fillers bridging the DMA window
USE_TILE_POS = False             # col-tile the E[x^2] matmuls at array cols 96+
X_FP8 = True                    # ship x as fp8e4m3

_PROGRAM = None


def _fold_weights(A, B_vec, eval_matrix, W_mlp, b_mlp):
    """Host-side weight folding in float64.

    Returns W1 [120, 6, 66] (cols: [1/T, 1/T, G]) and W2 [66, 6, 120]
    (rows: [v, 1-q, P^T]), both bf16, with t mapped as p*6 + a.
    """
    import ml_dtypes

    A64 = np.asarray(A, np.float64)
    Bv = np.asarray(B_vec, np.float64)
    G = np.empty((T, N), np.float64)
    r = Bv.copy()                       # r_k = B^T (A^T)^k
    for k in range(T):
        G[T - 1 - k] = r
        r = r @ A64.T
    P_mat = np.asarray(eval_matrix, np.float64) @ np.asarray(W_mlp, np.float64)
    v = np.asarray(eval_matrix, np.float64) @ np.asarray(b_mlp, np.float64)
    q = P_mat @ G.sum(axis=0)
    W1 = np.concatenate([np.full((T, 2), 1.0 / T), G], axis=1)
    W1 = W1.reshape(TT, NT, M1)                          # [120, 6, 66]
    W2 = np.concatenate([v[None, :], (1.0 - q)[None, :], P_mat.T], axis=0)
    W2 = W2.reshape(M1, TT, NT).transpose(0, 2, 1)       # [66, 6, 120]
    bf16 = ml_dtypes.bfloat16
    return (np.ascontiguousarray(W1).astype(bf16),
            np.ascontiguousarray(W2).astype(bf16))


def _build_program():
    from contextlib import ExitStack

    import concourse.tile as tile
    from concourse import bacc, mybir

    f32 = mybir.dt.float32
    bf16 = mybir.dt.bfloat16
    f16 = mybir.dt.float16
    f8 = mybir.dt.float8e4
    xdt = f8 if X_FP8 else bf16
    nc = bacc.Bacc("TRN2", target_bir_lowering=False, debug=False,
                   num_devices=N_CORES)

    w1 = nc.dram_tensor("w1", [TT, W1C], bf16, kind="ExternalInput")
    xs = nc.dram_tensor("xs", [TT, 2 * XB], xdt, kind="ExternalInput")
    w2 = nc.dram_tensor("w2", [M1, NT * TT], bf16, kind="ExternalInput")
    out = nc.dram_tensor("out", [B_PER_CORE, TT, NT, E], f16, kind="ExternalOutput")

    with tile.TileContext(nc) as tc, ExitStack() as ctx:
        consts = ctx.enter_context(tc.tile_pool(name="consts", bufs=1))
        xpool = ctx.enter_context(tc.tile_pool(name="xpool", bufs=1))
        sqpool = ctx.enter_context(tc.tile_pool(name="sqpool", bufs=1))
        stats = ctx.enter_context(tc.tile_pool(name="stats", bufs=1))
        opool = ctx.enter_context(tc.tile_pool(name="opool", bufs=1))
        psum_a = ctx.enter_context(tc.tile_pool(name="psum_a", bufs=1, space="PSUM"))
        psum_s = ctx.enter_context(tc.tile_pool(name="psum_s", bufs=1, space="PSUM"))
        psum_o = ctx.enter_context(tc.tile_pool(name="psum_o", bufs=1, space="PSUM"))

        # ---- input DMAs first: x0 gates phase A so it leads the HWDGE
        # ring; weights ride SWDGE concurrently
        x_sb = xpool.tile([TT, 2 * XB], xdt, name="x_sb")
        w1_sb = consts.tile([TT, W1C], bf16, name="w1_sb")
        w2_sb = consts.tile([M1, NT * TT], bf16, name="w2_sb")
        HXB = XB // 2
        nc.sync.dma_start(out=x_sb[:, 0:HXB], in_=xs[:, 0:HXB])
        nc.gpsimd.dma_start(out=w1_sb, in_=w1[:])
        nc.sync.dma_start(out=x_sb[:, HXB:XB], in_=xs[:, HXB:XB])
        nc.sync.dma_start(out=x_sb[:, XB:], in_=xs[:, XB:])
        nc.gpsimd.dma_start(out=w2_sb, in_=w2[:])

        # ---- tiny consts + ACT table preload + PE fillers ----
        eps_sb = consts.tile([1, 1], f32)
        wf = consts.tile([128, 272], bf16)
        nc.vector.memset(eps_sb, 1e-5)
        nc.vector.memset(wf, 1.0)
        dsq = consts.tile([1, 1], f32)
        nc.scalar.activation(dsq[:, :], eps_sb[:, :],
                             mybir.ActivationFunctionType.Sqrt,
                             bias=eps_sb[:, :])
        nc.scalar.square(dsq[:, :], eps_sb[:, :])
        pw = psum_o.tile([TT, EP], f32, tag="po_3", name="pw")
        for i in range(N_FILL):
            nc.tensor.matmul(pw[0:16, 0:256], lhsT=wf[:, 0:16],
                             rhs=wf[:, 16:272],
                             start=(i == 0), stop=(i == N_FILL - 1))

        xsq = [sqpool.tile([TT, XB], bf16, name=f"xsq_{b}")
               for b in range(B_PER_CORE)]
        p1s = [psum_a.tile([M1, EP], f32, tag=f"p1_{b}", name=f"p1_{b}")
               for b in range(B_PER_CORE)]
        pss = [psum_s.tile([128, EP], f32, tag=f"ps_{b}", name=f"ps_{b}")
               for b in range(B_PER_CORE)]
        SSR = 96 if USE_TILE_POS else 0   # psum row of the E[x^2] accumulator
        TP = (0, SSR) if USE_TILE_POS else None
        out_sb = opool.tile([TT, B_PER_CORE, NT, E], f16, name="out_sb")

        def phase_a(b):
            # U + mu accumulation (PE)
            xoff = b * XB
            for ti in range(NT):
                nc.tensor.matmul(p1s[b][:, :],
                                 lhsT=w1_sb[:, M1 * ti:M1 * ti + M1],
                                 rhs=x_sb[:, xoff + EP * ti:xoff + EP * (ti + 1)],
                                 start=(ti == 0), stop=(ti == NT - 1))

        def squares(b):
            # fp8 elementwise is slow on every engine; interleave DVE/ACT so
            # neither clogs the stats chain
            xoff = b * XB
            for h in range(2):
                cl, cr = XB // 2 * h, XB // 2 * (h + 1)
                if (b, h) in ((0, 0), (0, 1), (1, 1)):
                    nc.vector.tensor_mul(xsq[b][:, cl:cr],
                                         x_sb[:, xoff + cl:xoff + cr],
                                         x_sb[:, xoff + cl:xoff + cr])
                else:
                    nc.scalar.square(xsq[b][:, cl:cr],
                                     x_sb[:, xoff + cl:xoff + cr])

        def phase_ss(b):
            for ti in range(NT):
                nc.tensor.matmul(pss[b][SSR:SSR + 1, :], lhsT=w1_sb[:, 0:1],
                                 rhs=xsq[b][:, EP * ti:EP * (ti + 1)],
                                 start=(ti == 0), stop=(ti == NT - 1),
                                 tile_position=TP)

        rhs2s, musqs, vars_ = [], [], []

        def stats_pre(b):
            # needs only the U-group stop: mu^2 and the rhs2 bulk copy
            musq = stats.tile([1, EP], f32, name=f"musq_{b}")
            rhs2 = stats.tile([M1, EP], bf16, name=f"rhs2_{b}")
            musqs.append(musq)
            rhs2s.append(rhs2)
            nc.scalar.square(musq[:, :], p1s[b][0:1, :])
            nc.scalar.copy(rhs2[:, :], p1s[b][:, :])              # mu + U

        def stats_post(b):
            # needs the ss-group stop: var fold + sd
            var = stats.tile([1, EP], f32, name=f"var_{b}")
            vars_.append(var)
            nc.vector.tensor_sub(var[:, :], pss[b][SSR:SSR + 1, :],
                                 musqs[b][:, :])
            nc.scalar.activation(rhs2s[b][0:1, :], var[0:1, :],
                                 mybir.ActivationFunctionType.Sqrt,
                                 bias=eps_sb[0:1, :])             # sd

        def phase_c(b):
            rhs2 = rhs2s[b]
            for a in range(NT):
                po = psum_o.tile([TT, EP], f32, tag=f"po_{a % 4}",
                                 name=f"po_{b}_{a}")
                nc.tensor.matmul(po[:, :], lhsT=w2_sb[:, TT * a:TT * (a + 1)],
                                 rhs=rhs2[:, :], start=True, stop=True)
                if a % 2 == 0:
                    nc.vector.tensor_copy(out_sb[:, b, a, :], po[:, 0:E])
                else:
                    nc.scalar.copy(out_sb[:, b, a, :], po[:, 0:E])
                if b == 0 and a == 2:
                    nc.sync.dma_start(out=out[0][:, 0:3, :],
                                      in_=out_sb[:, 0, 0:3, :])
                if b == 1 and a == 2:
                    nc.sync.dma_start(out=out[1][:, 0:3, :],
                                      in_=out_sb[:, 1, 0:3, :])
                if b == 1 and a == 4:
                    nc.scalar.dma_start(out=out[1][:, 3:5, :],
                                        in_=out_sb[:, 1, 3:5, :])
            if b == 0:
                nc.sync.dma_start(out=out[0][:, 3:6, :],
                                  in_=out_sb[:, 0, 3:6, :])
            else:
                nc.sync.dma_start(out=out[1][:, 5:6, :],
                                  in_=out_sb[:, 1, 5:6, :])

        # pipelined issue order; PE queue: U0, ss0, U1, ss1, C0, C1.
        # stats_pre/post for both batches are issued before phase_c(0) so
        # the sqrt chain never queues behind phase-C copies on ACT
        phase_a(0)
        squares(0)
        phase_ss(0)
        stats_pre(0)
        squares(1)
        stats_post(0)
        phase_a(1)
        stats_pre(1)
        phase_ss(1)
        stats_post(1)
        phase_c(0)
        phase_c(1)

    nc.compile()
    return nc


def _get_program():
    global _PROGRAM
    if _PROGRAM is None:
        _PROGRAM = _build_program()
    return _PROGRAM


def _prepare_inputs(x_enc, A, B_vec, eval_matrix, W_mlp, b_mlp):
    import ml_dtypes

    bf16 = ml_dtypes.bfloat16
    xdt = ml_dtypes.float8_e4m3 if X_FP8 else bf16
    x = np.asarray(x_enc, np.float32)
    xp = np.zeros((BATCH, T, EP), np.float32)
    xp[:, :, :E] = x
    # t = p*6 + a layout: [B, 120, 6*322]
    xr = xp.reshape(BATCH, TT, XB).astype(xdt)
    W1, W2 = _fold_weights(A, B_vec, eval_matrix, W_mlp, b_mlp)
    w1_flat = np.ascontiguousarray(W1.reshape(TT, W1C))  # [120, 396]
    w2_flat = np.ascontiguousarray(W2.reshape(M1, NT * TT))
    ins = []
    for k in range(N_CORES):
        b0, b1 = k * B_PER_CORE, k * B_PER_CORE + 1
        xcat = np.concatenate([xr[b0], xr[b1]], axis=1)
        ins.append({"w1": w1_flat, "xs": np.ascontiguousarray(xcat),
                    "w2": w2_flat})
    return ins


def kernel(x_enc, A, B_vec, eval_matrix, W_mlp, b_mlp, affine_weight, affine_bias):
    from concourse.bass_utils import run_bass_kernel_spmd

    nc = _get_program()
    in_maps = _prepare_inputs(x_enc, A, B_vec, eval_matrix, W_mlp, b_mlp)
    res = run_bass_kernel_spmd(nc, in_maps, core_ids=list(range(N_CORES)))
    outs = [np.asarray(res.results[k]["out"]) for k in range(N_CORES)]
    full = np.concatenate(outs, axis=0)            # [16, 120, 6, 321] f16
    full = full.reshape(BATCH, T, E).astype(np.float32)
    return full
